# revision 9
# baseline (speedup 1.0000x reference)
"""Trainium2 Bass kernel for a dense transformer block (pre-LN, causal attention + FFN).

Contract: kernel(**inputs) takes the FULL inputs (B=128, T=256, C=384) and
returns the FULL output. Internally shards data-parallel over batch across
8 NeuronCores (16 sequences per core); weights are replicated.

Per-core pipeline, processed in batch PAIRS with stage-major emission and
cross-pair software pipelining (next pair's load/LN1/transpose/QKV is emitted
before the current pair's FFN) so the tensor engine never idles long enough
for the HAM clock gate to re-throttle:
  x -> LN1 -> PE transpose -> xnT (feature-major, f32r, pair-shared)
  QKV with N=512 moving operands; qT/kT feature-major per batch, v token-major
  scores^T = kT.T @ qT per head -> exp (no max subtraction; scores are O(1))
  causal mask via gpsimd affine_select
  av = v.T @ weiT; softmax sums via ones-column matmul (replicated rows so the
  reciprocal runs wide); odd heads col-tiled to partitions 64..128 so each
  attnT chunk holds a head pair -> proj runs K=128 matmuls
  proj + residual -> LN2 -> hnT; FFN1 (N=512) -> relu -> FFN2 -> out
All matmuls run in float32r (tf32-like, ~1e-4 rel err) at full PE rate.
"""

import os
import sys

for _p in ("/opt/trn_rl_repo", "/root/.axon_site/_ro/trn_rl_repo"):
    if os.path.isdir(_p) and _p not in sys.path:
        sys.path.append(_p)

import numpy as np

import concourse.bass as bass
import concourse.tile as tile
from concourse import bacc, mybir
from concourse.bass_utils import run_bass_kernel_spmd

f32 = mybir.dt.float32
f32r = mybir.dt.float32r
AF = mybir.ActivationFunctionType
ALU = mybir.AluOpType

N_CORES = 8
B, T, C = 128, 256, 384
H, D = 6, 64
F = 4 * C  # 1536
BPC = B // N_CORES  # 16 sequences per core
LN_EPS = 1e-5
ESC = float(C) ** -0.5

TRACE = False  # set by test harness to collect an NTFF profile
_CACHE = {}


def _build(bias_flags):
    """Build + compile the per-core program. bias_flags = (qb, kb, vb, bproj, b2)
    nonzero-ness; zero biases skip their K=1 fold-in matmuls."""
    has_qb, has_kb, has_vb, has_bp, has_b2 = bias_flags

    nc = bacc.Bacc("TRN2", target_bir_lowering=False, debug=False)

    x_d = nc.dram_tensor("x", (BPC, T, C), f32, kind="ExternalInput").ap()
    wq_d = nc.dram_tensor("wq", (C, C), f32r, kind="ExternalInput").ap()
    wk_d = nc.dram_tensor("wk", (C, C), f32r, kind="ExternalInput").ap()
    wv_d = nc.dram_tensor("wv", (C, C), f32r, kind="ExternalInput").ap()
    wp_d = nc.dram_tensor("wp", (H, D, C), f32r, kind="ExternalInput").ap()
    w1_d = nc.dram_tensor("w1", (C, F), f32r, kind="ExternalInput").ap()
    w2_d = nc.dram_tensor("w2", (F, C), f32r, kind="ExternalInput").ap()
    b1_d = nc.dram_tensor("b1", (F,), f32, kind="ExternalInput").ap()
    bias_d = nc.dram_tensor("biases", (5, C), f32r, kind="ExternalInput").ap()
    out_d = nc.dram_tensor("out", (BPC, T, C), f32, kind="ExternalOutput").ap()

    ident_d = nc.inline_tensor(np.eye(128, dtype=np.float32), name="identc").ap()
    ones_d = nc.inline_tensor(np.ones((128, 512), dtype=np.float32), name="onesc").ap()

    with tile.TileContext(nc) as tc:
        with tc.tile_pool(name="wpool", bufs=1) as wpool, \
             tc.tile_pool(name="pool", bufs=2) as pool, \
             tc.tile_pool(name="ppool", bufs=6, space="PSUM") as ppool:

            ident = wpool.tile([128, 128], f32)
            nc.sync.dma_start(ident[:], ident_d[:])
            ones = wpool.tile([128, 512], f32r)
            nc.sync.dma_start(ones[:], ones_d[:].bitcast(f32r))

            wq = wpool.tile([128, 3, C], f32r)
            nc.sync.dma_start(wq[:], wq_d.rearrange("(c p) j -> p c j", p=128))
            wk = wpool.tile([128, 3, C], f32r)
            nc.sync.dma_start(wk[:], wk_d.rearrange("(c p) j -> p c j", p=128))
            wv = wpool.tile([128, 3, C], f32r)
            nc.sync.dma_start(wv[:], wv_d.rearrange("(c p) j -> p c j", p=128))
            wp = wpool.tile([128, H, C], f32r)
            nc.sync.dma_start(wp[:64], wp_d.rearrange("h p j -> p h j"))
            w1 = wpool.tile([128, 3, F], f32r)
            nc.sync.dma_start(w1[:], w1_d.rearrange("(c p) j -> p c j", p=128))
            w2 = wpool.tile([128, 12, C], f32r)
            nc.sync.dma_start(w2[:], w2_d.rearrange("(m p) j -> p m j", p=128))
            b1 = wpool.tile([128, 12], f32)
            nc.sync.dma_start(b1[:], b1_d.rearrange("(m p) -> p m", p=128))
            eps_t = wpool.tile([128, 1], f32)
            nc.gpsimd.memset(eps_t[:], LN_EPS)
            biases = wpool.tile([128, 5, C], f32r)
            nc.sync.dma_start(biases[0:1], bias_d[None, :, :])
            qb, kb, vb, bpj, b2b = (biases[0:1, i, :] for i in range(5))

            def layernorm_stats(var2, in_pair):
                """in_pair [128, 2, C] -> var2 [128, kt, (mean, var)]."""
                for kt in range(2):
                    stats = pool.tile([128, 6], f32, tag="ln_stats", name="stats",
                                      bufs=6)
                    nc.vector.bn_stats(stats[:], in_pair[:, kt])
                    nc.vector.bn_aggr(var2[:, kt], stats[:])

            def layernorm_apply(out_pair, in_pair, var2):
                """out = (in - mean) * rsqrt(var + eps) for both kt tiles."""
                std = pool.tile([128, 2], f32, tag="ln_std", name="std", bufs=4)
                nc.scalar.activation(std[:], var2[:, :, 1], AF.Sqrt, bias=eps_t[:])
                rstd = pool.tile([128, 2], f32, tag="ln_rstd", name="rstd", bufs=4)
                nc.vector.reciprocal_approx_fast(rstd[:], std[:])
                for kt in range(2):
                    nc.vector.tensor_scalar(out_pair[:, kt], in_pair[:, kt],
                                            var2[:, kt, 0:1], rstd[:, kt:kt + 1],
                                            ALU.subtract, ALU.mult)

            def transpose_pair(dst, srcs, psname):
                """srcs: two [128, 2, C] f32 tiles -> dst [128, 3, 2T] f32r."""
                for bi, src in enumerate(srcs):
                    for c in range(3):
                        tp = ppool.tile([128, 2, 128], f32, tag="ps", name=psname)
                        for kt in range(2):
                            nc.tensor.transpose(tp[:, kt],
                                                src[:, kt, c * 128:(c + 1) * 128],
                                                ident[:])
                        nc.vector.tensor_copy(
                            dst[:, c, bi * T:(bi + 1) * T],
                            tp[:].rearrange("p a t -> p (a t)"))

            def load_ln_qkv(bp):
                """Load x, LN1, transpose, QKV for pair bp. Returns state dict."""
                pair = (2 * bp, 2 * bp + 1)
                xs, xns, var2s = [], [], []
                for b in pair:
                    x_t = pool.tile([128, 2, C], f32, tag="x", name="x_t")
                    for kt in range(2):
                        nc.sync.dma_start(x_t[:, kt],
                                          x_d[b, kt * 128:(kt + 1) * 128, :])
                    xs.append(x_t)
                for x_t in xs:
                    var2 = pool.tile([128, 2, 2], f32, tag="ln_var2", name="var2",
                                     bufs=4)
                    layernorm_stats(var2, x_t)
                    var2s.append(var2)
                for x_t, var2 in zip(xs, var2s):
                    xn = pool.tile([128, 2, C], f32, tag="xn", name="xn")
                    layernorm_apply(xn, x_t, var2)
                    xns.append(xn)

                xnT = pool.tile([128, 3, 2 * T], f32r, tag="xnT", name="xnT")
                transpose_pair(xnT, xns, "tp")

                qTs = [pool.tile([128, 3, T], f32r, tag="qT", name="qT")
                       for _ in range(2)]
                kTs = [pool.tile([128, 3, T], f32r, tag="kT", name="kT")
                       for _ in range(2)]
                for dsts, w, hb, hasb in ((qTs, wq, qb, has_qb),
                                          (kTs, wk, kb, has_kb)):
                    for m in range(3):
                        ps = ppool.tile([128, 2, T], f32, tag="ps", name="qk_ps")
                        flat = ps[:].rearrange("p a t -> p (a t)")
                        for c in range(3):
                            nc.tensor.matmul(flat, w[:, c, m * 128:(m + 1) * 128],
                                             xnT[:, c, :], start=(c == 0),
                                             stop=(c == 2 and not hasb))
                        if hasb:
                            nc.tensor.matmul(flat, hb[:, m * 128:(m + 1) * 128],
                                             ones[0:1, :], start=False, stop=True)
                        for bi in range(2):
                            nc.scalar.copy(dsts[bi][:, m, :], ps[:, bi])
                vs = [pool.tile([128, 2, H, D], f32r, tag="v", name="v")
                      for _ in range(2)]
                for bi in range(2):
                    for kt in range(2):
                        tk = 2 * bi + kt
                        ps = ppool.tile([128, C], f32, tag="ps", name="v_ps")
                        for c in range(3):
                            nc.tensor.matmul(
                                ps[:], xnT[:, c, tk * 128:(tk + 1) * 128],
                                wv[:, c, :], start=(c == 0),
                                stop=(c == 2 and not has_vb))
                        if has_vb:
                            nc.tensor.matmul(ps[:], ones[0:1, 0:128], vb,
                                             start=False, stop=True)
                        nc.vector.tensor_copy(
                            vs[bi][:, kt], ps[:].rearrange("p (h d) -> p h d", d=D))
                return {"pair": pair, "xs": xs, "qTs": qTs, "kTs": kTs, "vs": vs}

            def attention(st):
                """Causal attention per batch; attnT chunk ch holds heads
                (2ch, 2ch+1) on partitions (0..64, 64..128)."""
                qTs, kTs, vs = st["qTs"], st["kTs"], st["vs"]
                attnTs = [pool.tile([128, H, T], f32r, tag="attnT", name="attnT")
                          for _ in range(2)]
                for bi in range(2):
                    weiTs = []
                    for h in range(H):
                        po, ch = (h % 2) * 64, h // 2
                        sc = ppool.tile([128, 2, T], f32, tag="ps2", name="sc",
                                        bufs=2)
                        for kt in range(2):
                            nc.tensor.matmul(sc[:, kt],
                                             kTs[bi][po:po + 64, ch,
                                                     kt * 128:(kt + 1) * 128],
                                             qTs[bi][po:po + 64, ch, :],
                                             start=True, stop=True)
                        weiT = pool.tile([128, 2, T], f32r, tag="weiT",
                                         name="weiT", bufs=6)
                        nc.scalar.activation(weiT[:], sc[:], AF.Exp, scale=ESC)
                        for kt in range(2):
                            nc.gpsimd.affine_select(
                                out=weiT[:, kt], in_=weiT[:, kt],
                                compare_op=ALU.is_ge, fill=0.0,
                                base=-(kt * 128), pattern=[[1, T]],
                                channel_multiplier=-1)
                        weiTs.append(weiT)
                    for h in range(H):
                        weiT = weiTs[h]
                        av = ppool.tile([64, T], f32, tag="ps", name="av")
                        sm = ppool.tile([64, T], f32, tag="ps", name="sm")
                        for kt in range(2):
                            nc.tensor.matmul(av[:], vs[bi][:, kt, h, :],
                                             weiT[:, kt], start=(kt == 0),
                                             stop=(kt == 1))
                        for kt in range(2):
                            nc.tensor.matmul(sm[:], ones[:, 0:64],
                                             weiT[:, kt], start=(kt == 0),
                                             stop=(kt == 1))
                        rcp = pool.tile([64, T], f32, tag="rcp", name="rcp",
                                        bufs=3)
                        nc.vector.reciprocal_approx_fast(rcp[:], sm[:])
                        nc.vector.tensor_tensor(attnTs[bi][0:D, h, :],
                                                av[:], rcp[:], ALU.mult)
                st["attnTs"] = attnTs

            def proj_ln2(st):
                """proj + residual -> h, LN2 stats+apply, hn transposes -> hnT."""
                attnTs, xs = st["attnTs"], st["xs"]
                hs, hns, hvar2s = [], [], []
                for bi in range(2):
                    h_t = pool.tile([128, 2, C], f32, tag="h", name="h_t")
                    for kt in range(2):
                        ps = ppool.tile([128, C], f32, tag="ps", name="pr_ps")
                        for h in range(H):
                            nc.tensor.matmul(
                                ps[:], attnTs[bi][0:D, h, kt * 128:(kt + 1) * 128],
                                wp[0:64, h, :], start=(h == 0),
                                stop=(h == H - 1 and not has_bp))
                        if has_bp:
                            nc.tensor.matmul(ps[:], ones[0:1, 0:128], bpj,
                                             start=False, stop=True)
                        nc.vector.tensor_tensor(h_t[:, kt], ps[:], xs[bi][:, kt],
                                                ALU.add)
                    hs.append(h_t)
                    var2 = pool.tile([128, 2, 2], f32, tag="ln_hvar2",
                                     name="hvar2", bufs=4)
                    layernorm_stats(var2, h_t)
                    hvar2s.append(var2)
                for h_t, var2 in zip(hs, hvar2s):
                    hn = pool.tile([128, 2, C], f32, tag="hn", name="hn")
                    layernorm_apply(hn, h_t, var2)
                    hns.append(hn)
                hnT = pool.tile([128, 3, 2 * T], f32r, tag="hnT", name="hnT")
                transpose_pair(hnT, hns, "tph")
                st["hs"], st["hnT"] = hs, hnT

            def ffn(st):
                pair, hs, hnT = st["pair"], st["hs"], st["hnT"]
                h1T = pool.tile([128, 12, 2 * T], f32r, tag="h1T", name="h1T",
                                bufs=1)
                for mf in range(12):
                    ps = ppool.tile([128, 2, T], f32, tag="ps", name="f1_ps")
                    flat = ps[:].rearrange("p a t -> p (a t)")
                    for c in range(3):
                        nc.tensor.matmul(flat, w1[:, c, mf * 128:(mf + 1) * 128],
                                         hnT[:, c, :], start=(c == 0),
                                         stop=(c == 2))
                    nc.scalar.activation(h1T[:, mf, :], flat, AF.Relu,
                                         bias=b1[:, mf:mf + 1])
                for bi, b in enumerate(pair):
                    out_t = pool.tile([128, 2, C], f32, tag="out", name="out_t")
                    for kt in range(2):
                        tk = 2 * bi + kt
                        ps = ppool.tile([128, C], f32, tag="ps", name="f2_ps")
                        for mf in range(12):
                            nc.tensor.matmul(
                                ps[:], h1T[:, mf, tk * 128:(tk + 1) * 128],
                                w2[:, mf, :], start=(mf == 0),
                                stop=(mf == 11 and not has_b2))
                        if has_b2:
                            nc.tensor.matmul(ps[:], ones[0:1, 0:128], b2b,
                                             start=False, stop=True)
                        nc.vector.tensor_tensor(out_t[:, kt], ps[:],
                                                hs[bi][:, kt], ALU.add)
                        nc.sync.dma_start(out_d[b, kt * 128:(kt + 1) * 128, :],
                                          out_t[:, kt])

            # Software pipeline: emit next pair's load/LN1/QKV before this
            # pair's FFN so the PE never idles at pair boundaries.
            NP = BPC // 2
            pending = load_ln_qkv(0)
            for bp in range(NP):
                st = pending
                attention(st)
                proj_ln2(st)
                if bp + 1 < NP:
                    pending = load_ln_qkv(bp + 1)
                ffn(st)

    nc.compile()
    return nc


def kernel(x, Wq, Wk, Wv, Wproj, bproj, W1, b1, W2, b2, ln1_g, ln1_b, ln2_g, ln2_b):
    x = np.asarray(x, dtype=np.float32)
    Wq = np.asarray(Wq, dtype=np.float32)
    Wk = np.asarray(Wk, dtype=np.float32)
    Wv = np.asarray(Wv, dtype=np.float32)
    Wproj = np.asarray(Wproj, dtype=np.float32)
    bproj = np.asarray(bproj, dtype=np.float32)
    W1 = np.asarray(W1, dtype=np.float32)
    b1 = np.asarray(b1, dtype=np.float32)
    W2 = np.asarray(W2, dtype=np.float32)
    b2 = np.asarray(b2, dtype=np.float32)
    ln1_g = np.asarray(ln1_g, dtype=np.float32)
    ln1_b = np.asarray(ln1_b, dtype=np.float32)
    ln2_g = np.asarray(ln2_g, dtype=np.float32)
    ln2_b = np.asarray(ln2_b, dtype=np.float32)

    # Fold LN gains into the consuming weights; LN biases become extra input-side
    # biases folded through the weights (zero for the standard init).
    wq_h = np.ascontiguousarray(Wq.transpose(1, 0, 2).reshape(C, C) * ln1_g[:, None])
    wk_h = np.ascontiguousarray(Wk.transpose(1, 0, 2).reshape(C, C) * ln1_g[:, None])
    wv_h = np.ascontiguousarray(Wv.transpose(1, 0, 2).reshape(C, C) * ln1_g[:, None])
    qb_h = ln1_b @ wq_h
    kb_h = ln1_b @ wk_h
    vb_h = ln1_b @ wv_h
    wp_h = np.ascontiguousarray(Wproj.reshape(H, D, C))
    w1_h = np.ascontiguousarray(W1 * ln2_g[:, None])
    b1_h = np.ascontiguousarray(b1 + ln2_b @ w1_h)
    w2_h = np.ascontiguousarray(W2)
    biases_h = np.ascontiguousarray(np.stack([qb_h, kb_h, vb_h, bproj, b2]))

    flags = tuple(bool(np.any(v)) for v in (qb_h, kb_h, vb_h, bproj, b2))
    if flags not in _CACHE:
        _CACHE[flags] = _build(flags)
    nc = _CACHE[flags]

    shared = {"wq": wq_h, "wk": wk_h, "wv": wv_h, "wp": wp_h,
              "w1": w1_h, "w2": w2_h, "b1": b1_h, "biases": biases_h}
    in_maps = [{"x": np.ascontiguousarray(x[c * BPC:(c + 1) * BPC]), **shared}
               for c in range(N_CORES)]

    res = run_bass_kernel_spmd(nc, in_maps, list(range(N_CORES)), trace=TRACE)
    if TRACE:
        kernel.last_results = res
    return np.concatenate([res.results[c]["out"] for c in range(N_CORES)], axis=0)


# revision 10
# speedup vs baseline: 1.2808x; 1.2808x over previous
"""Trainium2 Bass kernel for a dense transformer block (pre-LN, causal attention + FFN).

Contract: kernel(**inputs) takes the FULL inputs (B=128, T=256, C=384) and
returns the FULL output. Internally shards data-parallel over batch across
8 NeuronCores (16 sequences per core); weights are replicated.

Per-core pipeline, processed in batch PAIRS with stage-major emission and
cross-pair software pipelining (next pair's load/LN1/transpose/QKV is emitted
before the current pair's FFN) so the tensor engine never idles long enough
for the HAM clock gate to re-throttle:
  x -> LN1 -> PE transpose -> xnT (feature-major, f32r, pair-shared)
  QKV with N=512 moving operands; qT/kT feature-major per batch, v token-major
  scores^T = kT.T @ qT per head -> exp (no max subtraction; scores are O(1))
  causal mask via gpsimd affine_select
  av = v.T @ weiT; softmax sums via ones-column matmul (replicated rows so the
  reciprocal runs wide); odd heads col-tiled to partitions 64..128 so each
  attnT chunk holds a head pair -> proj runs K=128 matmuls
  proj + residual -> LN2 -> hnT; FFN1 (N=512) -> relu -> FFN2 -> out
All matmuls run in float32r (tf32-like, ~1e-4 rel err) at full PE rate.
"""

import os
import sys

for _p in ("/opt/trn_rl_repo", "/root/.axon_site/_ro/trn_rl_repo"):
    if os.path.isdir(_p) and _p not in sys.path:
        sys.path.append(_p)

import numpy as np

import concourse.bass as bass
import concourse.tile as tile
from concourse import bacc, mybir
from concourse.bass_utils import run_bass_kernel_spmd

f32 = mybir.dt.float32
f32r = mybir.dt.float32r
AF = mybir.ActivationFunctionType
ALU = mybir.AluOpType

N_CORES = 8
B, T, C = 128, 256, 384
H, D = 6, 64
F = 4 * C  # 1536
BPC = B // N_CORES  # 16 sequences per core
LN_EPS = 1e-5
ESC = float(C) ** -0.5

TRACE = False  # set by test harness to collect an NTFF profile
_CACHE = {}


def _build(bias_flags):
    """Build + compile the per-core program. bias_flags = (qb, kb, vb, bproj, b2)
    nonzero-ness; zero biases skip their K=1 fold-in matmuls."""
    has_qb, has_kb, has_vb, has_bp, has_b2 = bias_flags

    nc = bacc.Bacc("TRN2", target_bir_lowering=False, debug=False)

    x_d = nc.dram_tensor("x", (BPC, T, C), f32, kind="ExternalInput").ap()
    wq_d = nc.dram_tensor("wq", (C, C), f32r, kind="ExternalInput").ap()
    wk_d = nc.dram_tensor("wk", (C, C), f32r, kind="ExternalInput").ap()
    wv_d = nc.dram_tensor("wv", (C, C), f32r, kind="ExternalInput").ap()
    wp_d = nc.dram_tensor("wp", (H, D, C), f32r, kind="ExternalInput").ap()
    w1_d = nc.dram_tensor("w1", (C, F), f32r, kind="ExternalInput").ap()
    w2_d = nc.dram_tensor("w2", (F, C), f32r, kind="ExternalInput").ap()
    b1_d = nc.dram_tensor("b1", (F,), f32, kind="ExternalInput").ap()
    bias_d = nc.dram_tensor("biases", (5, C), f32r, kind="ExternalInput").ap()
    out_d = nc.dram_tensor("out", (BPC, T, C), f32, kind="ExternalOutput").ap()

    ident_d = nc.inline_tensor(np.eye(128, dtype=np.float32), name="identc").ap()
    ones_d = nc.inline_tensor(np.ones((128, 512), dtype=np.float32), name="onesc").ap()

    with tile.TileContext(nc) as tc:
        with tc.tile_pool(name="wpool", bufs=1) as wpool, \
             tc.tile_pool(name="pool", bufs=2) as pool, \
             tc.tile_pool(name="ppool", bufs=8, space="PSUM") as ppool:

            ident = wpool.tile([128, 128], f32)
            nc.sync.dma_start(ident[:], ident_d[:])
            ones = wpool.tile([128, 512], f32r)
            nc.sync.dma_start(ones[:], ones_d[:].bitcast(f32r))

            wq = wpool.tile([128, 3, C], f32r)
            nc.sync.dma_start(wq[:], wq_d.rearrange("(c p) j -> p c j", p=128))
            wk = wpool.tile([128, 3, C], f32r)
            nc.sync.dma_start(wk[:], wk_d.rearrange("(c p) j -> p c j", p=128))
            wv = wpool.tile([128, 3, C], f32r)
            nc.sync.dma_start(wv[:], wv_d.rearrange("(c p) j -> p c j", p=128))
            wp = wpool.tile([128, H, C], f32r)
            nc.sync.dma_start(wp[:64], wp_d.rearrange("h p j -> p h j"))
            w1 = wpool.tile([128, 3, F], f32r)
            nc.sync.dma_start(w1[:], w1_d.rearrange("(c p) j -> p c j", p=128))
            w2 = wpool.tile([128, 12, C], f32r)
            nc.sync.dma_start(w2[:], w2_d.rearrange("(m p) j -> p m j", p=128))
            b1 = wpool.tile([128, 12], f32)
            nc.sync.dma_start(b1[:], b1_d.rearrange("(m p) -> p m", p=128))
            eps_t = wpool.tile([128, 1], f32)
            nc.gpsimd.memset(eps_t[:], LN_EPS)
            biases = wpool.tile([128, 5, C], f32r)
            nc.sync.dma_start(biases[0:1], bias_d[None, :, :])
            qb, kb, vb, bpj, b2b = (biases[0:1, i, :] for i in range(5))

            def layernorm_stats(var2, in_pair):
                """in_pair [128, 2, C] -> var2 [128, kt, (mean, var)]."""
                for kt in range(2):
                    stats = pool.tile([128, 6], f32, tag="ln_stats", name="stats",
                                      bufs=6)
                    nc.vector.bn_stats(stats[:], in_pair[:, kt])
                    nc.vector.bn_aggr(var2[:, kt], stats[:])

            def layernorm_apply(out_pair, in_pair, var2):
                """out = (in - mean) * rsqrt(var + eps) for both kt tiles."""
                std = pool.tile([128, 2], f32, tag="ln_std", name="std", bufs=4)
                nc.scalar.activation(std[:], var2[:, :, 1], AF.Sqrt, bias=eps_t[:])
                rstd = pool.tile([128, 2], f32, tag="ln_rstd", name="rstd", bufs=4)
                nc.vector.reciprocal_approx_fast(rstd[:], std[:])
                for kt in range(2):
                    nc.vector.tensor_scalar(out_pair[:, kt], in_pair[:, kt],
                                            var2[:, kt, 0:1], rstd[:, kt:kt + 1],
                                            ALU.subtract, ALU.mult)

            def transpose_pair(dst, srcs, psname):
                """srcs: two [128, 2, C] f32 tiles -> dst [128, 3, 2T] f32r."""
                for bi, src in enumerate(srcs):
                    for c in range(3):
                        tp = ppool.tile([128, 2, 128], f32, tag="ps", name=psname)
                        for kt in range(2):
                            nc.tensor.transpose(tp[:, kt],
                                                src[:, kt, c * 128:(c + 1) * 128],
                                                ident[:])
                        nc.vector.tensor_copy(
                            dst[:, c, bi * T:(bi + 1) * T],
                            tp[:].rearrange("p a t -> p (a t)"))

            def ln1_load(bp):
                """Load x and run LN1 for pair bp (DVE/ACT only, no PE work)."""
                pair = (2 * bp, 2 * bp + 1)
                xs, xns, var2s = [], [], []
                for b in pair:
                    x_t = pool.tile([128, 2, C], f32, tag="x", name="x_t", bufs=4)
                    for kt in range(2):
                        nc.sync.dma_start(x_t[:, kt],
                                          x_d[b, kt * 128:(kt + 1) * 128, :])
                    xs.append(x_t)
                for x_t in xs:
                    var2 = pool.tile([128, 2, 2], f32, tag="ln_var2", name="var2",
                                     bufs=4)
                    layernorm_stats(var2, x_t)
                    var2s.append(var2)
                for x_t, var2 in zip(xs, var2s):
                    xn = pool.tile([128, 2, C], f32, tag="xn", name="xn")
                    layernorm_apply(xn, x_t, var2)
                    xns.append(xn)
                return {"pair": pair, "xs": xs, "xns": xns}

            def tp_qkv(st):
                """Transpose xn -> xnT and run QKV matmuls (PE-heavy)."""
                xns = st.pop("xns")
                xnT = pool.tile([128, 3, 2 * T], f32r, tag="xnT", name="xnT")
                transpose_pair(xnT, xns, "tp")

                qTs = [pool.tile([128, 3, T], f32r, tag="qT", name="qT")
                       for _ in range(2)]
                kTs = [pool.tile([128, 3, T], f32r, tag="kT", name="kT")
                       for _ in range(2)]
                for dsts, w, hb, hasb in ((qTs, wq, qb, has_qb),
                                          (kTs, wk, kb, has_kb)):
                    for m in range(3):
                        ps = ppool.tile([128, 2, T], f32, tag="ps", name="qk_ps")
                        flat = ps[:].rearrange("p a t -> p (a t)")
                        for c in range(3):
                            nc.tensor.matmul(flat, w[:, c, m * 128:(m + 1) * 128],
                                             xnT[:, c, :], start=(c == 0),
                                             stop=(c == 2 and not hasb))
                        if hasb:
                            nc.tensor.matmul(flat, hb[:, m * 128:(m + 1) * 128],
                                             ones[0:1, :], start=False, stop=True)
                        for bi in range(2):
                            nc.scalar.copy(dsts[bi][:, m, :], ps[:, bi])
                vs = [pool.tile([128, 2, H, D], f32r, tag="v", name="v")
                      for _ in range(2)]
                for bi in range(2):
                    for kt in range(2):
                        tk = 2 * bi + kt
                        ps = ppool.tile([128, C], f32, tag="ps", name="v_ps")
                        for c in range(3):
                            nc.tensor.matmul(
                                ps[:], xnT[:, c, tk * 128:(tk + 1) * 128],
                                wv[:, c, :], start=(c == 0),
                                stop=(c == 2 and not has_vb))
                        if has_vb:
                            nc.tensor.matmul(ps[:], ones[0:1, 0:128], vb,
                                             start=False, stop=True)
                        nc.vector.tensor_copy(
                            vs[bi][:, kt], ps[:].rearrange("p (h d) -> p h d", d=D))
                st.update(qTs=qTs, kTs=kTs, vs=vs)
                return st

            def attention(st):
                """Causal attention per batch; attnT chunk ch holds heads
                (2ch, 2ch+1) on partitions (0..64, 64..128)."""
                qTs, kTs, vs = st["qTs"], st["kTs"], st["vs"]
                attnTs = [pool.tile([128, H, T], f32r, tag="attnT", name="attnT")
                          for _ in range(2)]
                for bi in range(2):
                    weiTs = []
                    for h in range(H):
                        po, ch = (h % 2) * 64, h // 2
                        sc = ppool.tile([128, 2, T], f32, tag="ps", name="sc")
                        for kt in range(2):
                            nc.tensor.matmul(sc[:, kt],
                                             kTs[bi][po:po + 64, ch,
                                                     kt * 128:(kt + 1) * 128],
                                             qTs[bi][po:po + 64, ch, :],
                                             start=True, stop=True)
                        weiT = pool.tile([128, 2, T], f32r, tag="weiT",
                                         name="weiT", bufs=6)
                        nc.scalar.activation(weiT[:], sc[:], AF.Exp, scale=ESC)
                        for kt in range(2):
                            nc.gpsimd.affine_select(
                                out=weiT[:, kt], in_=weiT[:, kt],
                                compare_op=ALU.is_ge, fill=0.0,
                                base=-(kt * 128), pattern=[[1, T]],
                                channel_multiplier=-1)
                        weiTs.append(weiT)
                    for h in range(H):
                        weiT = weiTs[h]
                        av = ppool.tile([64, T], f32, tag="ps", name="av")
                        sm = ppool.tile([64, T], f32, tag="ps", name="sm")
                        for kt in range(2):
                            nc.tensor.matmul(av[:], vs[bi][:, kt, h, :],
                                             weiT[:, kt], start=(kt == 0),
                                             stop=(kt == 1))
                        for kt in range(2):
                            nc.tensor.matmul(sm[:], ones[:, 0:64],
                                             weiT[:, kt], start=(kt == 0),
                                             stop=(kt == 1))
                        rcp = pool.tile([64, T], f32, tag="rcp", name="rcp",
                                        bufs=3)
                        nc.vector.reciprocal_approx_fast(rcp[:], sm[:])
                        nc.vector.tensor_tensor(attnTs[bi][0:D, h, :],
                                                av[:], rcp[:], ALU.mult)
                st["attnTs"] = attnTs

            def proj_stats(st):
                """proj + residual -> h, LN2 stats (PE: proj matmuls)."""
                attnTs, xs = st["attnTs"], st["xs"]
                hs, hvar2s = [], []
                for bi in range(2):
                    h_t = pool.tile([128, 2, C], f32, tag="h", name="h_t")
                    for kt in range(2):
                        ps = ppool.tile([128, C], f32, tag="ps", name="pr_ps")
                        for h in range(H):
                            nc.tensor.matmul(
                                ps[:], attnTs[bi][0:D, h, kt * 128:(kt + 1) * 128],
                                wp[0:64, h, :], start=(h == 0),
                                stop=(h == H - 1 and not has_bp))
                        if has_bp:
                            nc.tensor.matmul(ps[:], ones[0:1, 0:128], bpj,
                                             start=False, stop=True)
                        nc.vector.tensor_tensor(h_t[:, kt], ps[:], xs[bi][:, kt],
                                                ALU.add)
                    hs.append(h_t)
                    var2 = pool.tile([128, 2, 2], f32, tag="ln_hvar2",
                                     name="hvar2", bufs=4)
                    layernorm_stats(var2, h_t)
                    hvar2s.append(var2)
                st["hs"], st["hvar2s"] = hs, hvar2s

            def apply_hnT(st):
                """LN2 apply + hn transposes -> hnT."""
                hns = []
                for h_t, var2 in zip(st["hs"], st.pop("hvar2s")):
                    hn = pool.tile([128, 2, C], f32, tag="hn", name="hn")
                    layernorm_apply(hn, h_t, var2)
                    hns.append(hn)
                hnT = pool.tile([128, 3, 2 * T], f32r, tag="hnT", name="hnT")
                transpose_pair(hnT, hns, "tph")
                st["hnT"] = hnT

            def ffn(st):
                pair, hs, hnT = st["pair"], st["hs"], st["hnT"]
                h1T = pool.tile([128, 12, 2 * T], f32r, tag="h1T", name="h1T",
                                bufs=1)
                for mf in range(12):
                    ps = ppool.tile([128, 2, T], f32, tag="ps", name="f1_ps")
                    flat = ps[:].rearrange("p a t -> p (a t)")
                    for c in range(3):
                        nc.tensor.matmul(flat, w1[:, c, mf * 128:(mf + 1) * 128],
                                         hnT[:, c, :], start=(c == 0),
                                         stop=(c == 2))
                    nc.scalar.activation(h1T[:, mf, :], flat, AF.Relu,
                                         bias=b1[:, mf:mf + 1])
                for bi, b in enumerate(pair):
                    out_t = pool.tile([128, 2, C], f32, tag="out", name="out_t")
                    for kt in range(2):
                        tk = 2 * bi + kt
                        ps = ppool.tile([128, C], f32, tag="ps", name="f2_ps")
                        for mf in range(12):
                            nc.tensor.matmul(
                                ps[:], h1T[:, mf, tk * 128:(tk + 1) * 128],
                                w2[:, mf, :], start=(mf == 0),
                                stop=(mf == 11 and not has_b2))
                        if has_b2:
                            nc.tensor.matmul(ps[:], ones[0:1, 0:128], b2b,
                                             start=False, stop=True)
                        nc.vector.tensor_tensor(out_t[:, kt], ps[:],
                                                hs[bi][:, kt], ALU.add)
                        nc.sync.dma_start(out_d[b, kt * 128:(kt + 1) * 128, :],
                                          out_t[:, kt])

            # Software pipeline: the next pair's LN1 runs during this pair's
            # attention, and its transposes+QKV (PE work) fill the LN2 chain
            # latency, so the PE never idles long enough to re-throttle.
            NP = BPC // 2
            st = tp_qkv(ln1_load(0))
            for bp in range(NP):
                attention(st)
                nxt = ln1_load(bp + 1) if bp + 1 < NP else None
                proj_stats(st)
                if nxt is not None:
                    nxt = tp_qkv(nxt)
                apply_hnT(st)
                ffn(st)
                st = nxt

    nc.compile()
    return nc


def kernel(x, Wq, Wk, Wv, Wproj, bproj, W1, b1, W2, b2, ln1_g, ln1_b, ln2_g, ln2_b):
    x = np.asarray(x, dtype=np.float32)
    Wq = np.asarray(Wq, dtype=np.float32)
    Wk = np.asarray(Wk, dtype=np.float32)
    Wv = np.asarray(Wv, dtype=np.float32)
    Wproj = np.asarray(Wproj, dtype=np.float32)
    bproj = np.asarray(bproj, dtype=np.float32)
    W1 = np.asarray(W1, dtype=np.float32)
    b1 = np.asarray(b1, dtype=np.float32)
    W2 = np.asarray(W2, dtype=np.float32)
    b2 = np.asarray(b2, dtype=np.float32)
    ln1_g = np.asarray(ln1_g, dtype=np.float32)
    ln1_b = np.asarray(ln1_b, dtype=np.float32)
    ln2_g = np.asarray(ln2_g, dtype=np.float32)
    ln2_b = np.asarray(ln2_b, dtype=np.float32)

    # Fold LN gains into the consuming weights; LN biases become extra input-side
    # biases folded through the weights (zero for the standard init).
    wq_h = np.ascontiguousarray(Wq.transpose(1, 0, 2).reshape(C, C) * ln1_g[:, None])
    wk_h = np.ascontiguousarray(Wk.transpose(1, 0, 2).reshape(C, C) * ln1_g[:, None])
    wv_h = np.ascontiguousarray(Wv.transpose(1, 0, 2).reshape(C, C) * ln1_g[:, None])
    qb_h = ln1_b @ wq_h
    kb_h = ln1_b @ wk_h
    vb_h = ln1_b @ wv_h
    wp_h = np.ascontiguousarray(Wproj.reshape(H, D, C))
    w1_h = np.ascontiguousarray(W1 * ln2_g[:, None])
    b1_h = np.ascontiguousarray(b1 + ln2_b @ w1_h)
    w2_h = np.ascontiguousarray(W2)
    biases_h = np.ascontiguousarray(np.stack([qb_h, kb_h, vb_h, bproj, b2]))

    flags = tuple(bool(np.any(v)) for v in (qb_h, kb_h, vb_h, bproj, b2))
    if flags not in _CACHE:
        _CACHE[flags] = _build(flags)
    nc = _CACHE[flags]

    shared = {"wq": wq_h, "wk": wk_h, "wv": wv_h, "wp": wp_h,
              "w1": w1_h, "w2": w2_h, "b1": b1_h, "biases": biases_h}
    in_maps = [{"x": np.ascontiguousarray(x[c * BPC:(c + 1) * BPC]), **shared}
               for c in range(N_CORES)]

    res = run_bass_kernel_spmd(nc, in_maps, list(range(N_CORES)), trace=TRACE)
    if TRACE:
        kernel.last_results = res
    return np.concatenate([res.results[c]["out"] for c in range(N_CORES)], axis=0)


# revision 15
# speedup vs baseline: 1.2940x; 1.0103x over previous
"""Trainium2 Bass kernel for a dense transformer block (pre-LN, causal attention + FFN).

Contract: kernel(**inputs) takes the FULL inputs (B=128, T=256, C=384) and
returns the FULL output. Internally shards data-parallel over batch across
8 NeuronCores (16 sequences per core); weights are replicated.

Per-core pipeline, processed in batch PAIRS with stage-major emission and
cross-pair software pipelining (next pair's load/LN1/transpose/QKV is emitted
before the current pair's FFN) so the tensor engine never idles long enough
for the HAM clock gate to re-throttle:
  x -> LN1 -> PE transpose -> xnT (feature-major, f32r, pair-shared)
  QKV with N=512 moving operands; qT/kT feature-major per batch, v token-major
  scores^T = kT.T @ qT per head -> exp (no max subtraction; scores are O(1))
  causal mask via gpsimd affine_select
  av = v.T @ weiT; softmax sums via ones-column matmul (replicated rows so the
  reciprocal runs wide); odd heads col-tiled to partitions 64..128 so each
  attnT chunk holds a head pair -> proj runs K=128 matmuls
  proj + residual -> LN2 -> hnT; FFN1 (N=512) -> relu -> FFN2 -> out
All matmuls run in float32r (tf32-like, ~1e-4 rel err) at full PE rate.
"""

import os
import sys

for _p in ("/opt/trn_rl_repo", "/root/.axon_site/_ro/trn_rl_repo"):
    if os.path.isdir(_p) and _p not in sys.path:
        sys.path.append(_p)

import numpy as np

import concourse.bass as bass
import concourse.tile as tile
from concourse import bacc, mybir
from concourse.bass_utils import run_bass_kernel_spmd

f32 = mybir.dt.float32
f32r = mybir.dt.float32r
AF = mybir.ActivationFunctionType
ALU = mybir.AluOpType

N_CORES = 8
B, T, C = 128, 256, 384
H, D = 6, 64
F = 4 * C  # 1536
BPC = B // N_CORES  # 16 sequences per core
LN_EPS = 1e-5
ESC = float(C) ** -0.5

TRACE = False  # set by test harness to collect an NTFF profile
_CACHE = {}


def _build(bias_flags):
    """Build + compile the per-core program. bias_flags = (qb, kb, vb, bproj, b2)
    nonzero-ness; zero biases skip their K=1 fold-in matmuls."""
    has_qb, has_kb, has_vb, has_bp, has_b2 = bias_flags

    nc = bacc.Bacc("TRN2", target_bir_lowering=False, debug=False)

    x_d = nc.dram_tensor("x", (BPC, T, C), f32, kind="ExternalInput").ap()
    wq_d = nc.dram_tensor("wq", (C, C), f32r, kind="ExternalInput").ap()
    wk_d = nc.dram_tensor("wk", (C, C), f32r, kind="ExternalInput").ap()
    wv_d = nc.dram_tensor("wv", (C, C), f32r, kind="ExternalInput").ap()
    wp_d = nc.dram_tensor("wp", (H, D, C), f32r, kind="ExternalInput").ap()
    w1_d = nc.dram_tensor("w1", (C, F), f32r, kind="ExternalInput").ap()
    w2_d = nc.dram_tensor("w2", (F, C), f32r, kind="ExternalInput").ap()
    b1_d = nc.dram_tensor("b1", (F,), f32, kind="ExternalInput").ap()
    bias_d = nc.dram_tensor("biases", (5, C), f32r, kind="ExternalInput").ap()
    out_d = nc.dram_tensor("out", (BPC, T, C), f32, kind="ExternalOutput").ap()

    ident_d = nc.inline_tensor(np.eye(128, dtype=np.float32), name="identc").ap()
    ones_d = nc.inline_tensor(np.ones((128, 512), dtype=np.float32), name="onesc").ap()

    with tile.TileContext(nc) as tc:
        with tc.tile_pool(name="wpool", bufs=1) as wpool, \
             tc.tile_pool(name="pool", bufs=2) as pool, \
             tc.tile_pool(name="ppool", bufs=8, space="PSUM") as ppool:

            ident = wpool.tile([128, 128], f32)
            nc.sync.dma_start(ident[:], ident_d[:])
            ones = wpool.tile([128, 512], f32r)
            nc.sync.dma_start(ones[:], ones_d[:].bitcast(f32r))

            wq = wpool.tile([128, 3, C], f32r)
            nc.sync.dma_start(wq[:], wq_d.rearrange("(c p) j -> p c j", p=128))
            wk = wpool.tile([128, 3, C], f32r)
            nc.sync.dma_start(wk[:], wk_d.rearrange("(c p) j -> p c j", p=128))
            wv = wpool.tile([128, 3, C], f32r)
            nc.sync.dma_start(wv[:], wv_d.rearrange("(c p) j -> p c j", p=128))
            wp = wpool.tile([128, H, C], f32r)
            nc.sync.dma_start(wp[:64], wp_d.rearrange("h p j -> p h j"))
            w1 = wpool.tile([128, 3, F], f32r)
            nc.sync.dma_start(w1[:], w1_d.rearrange("(c p) j -> p c j", p=128))
            w2 = wpool.tile([128, 12, C], f32r)
            nc.sync.dma_start(w2[:], w2_d.rearrange("(m p) j -> p m j", p=128))
            b1 = wpool.tile([128, 12], f32)
            nc.sync.dma_start(b1[:], b1_d.rearrange("(m p) -> p m", p=128))
            eps_t = wpool.tile([128, 1], f32)
            nc.gpsimd.memset(eps_t[:], LN_EPS)
            biases = wpool.tile([128, 5, C], f32r)
            nc.sync.dma_start(biases[0:1], bias_d[None, :, :])
            qb, kb, vb, bpj, b2b = (biases[0:1, i, :] for i in range(5))

            def layernorm_stats(var2, in_pair):
                """in_pair [128, 2, C] -> var2 [128, kt, (mean, var)]."""
                for kt in range(2):
                    stats = pool.tile([128, 6], f32, tag="ln_stats", name="stats",
                                      bufs=6)
                    nc.vector.bn_stats(stats[:], in_pair[:, kt])
                    nc.vector.bn_aggr(var2[:, kt], stats[:])

            def layernorm_apply(out_pair, in_pair, var2):
                """out = (in - mean) * rsqrt(var + eps) for both kt tiles."""
                std = pool.tile([128, 2], f32, tag="ln_std", name="std", bufs=4)
                nc.scalar.activation(std[:], var2[:, :, 1], AF.Sqrt, bias=eps_t[:])
                rstd = pool.tile([128, 2], f32, tag="ln_rstd", name="rstd", bufs=4)
                nc.vector.reciprocal_approx_fast(rstd[:], std[:])
                for kt in range(2):
                    nc.vector.tensor_scalar(out_pair[:, kt], in_pair[:, kt],
                                            var2[:, kt, 0:1], rstd[:, kt:kt + 1],
                                            ALU.subtract, ALU.mult)

            def transpose_pair(dst, srcs, psname):
                """srcs: two [128, 2, C] f32 tiles -> dst [128, 3, 2T] f32r."""
                for bi, src in enumerate(srcs):
                    for c in range(3):
                        tp = ppool.tile([128, 2, 128], f32, tag="ps", name=psname)
                        for kt in range(2):
                            nc.tensor.transpose(tp[:, kt],
                                                src[:, kt, c * 128:(c + 1) * 128],
                                                ident[:])
                        nc.vector.tensor_copy(
                            dst[:, c, bi * T:(bi + 1) * T],
                            tp[:].rearrange("p a t -> p (a t)"))

            def ln1_load(bp):
                """Load x and run LN1 for pair bp (DVE/ACT only, no PE work)."""
                pair = (2 * bp, 2 * bp + 1)
                xs, xns, var2s = [], [], []
                for b in pair:
                    x_t = pool.tile([128, 2, C], f32, tag="x", name="x_t", bufs=4)
                    for kt in range(2):
                        nc.sync.dma_start(x_t[:, kt],
                                          x_d[b, kt * 128:(kt + 1) * 128, :])
                    xs.append(x_t)
                for x_t in xs:
                    var2 = pool.tile([128, 2, 2], f32, tag="ln_var2", name="var2",
                                     bufs=4)
                    layernorm_stats(var2, x_t)
                    var2s.append(var2)
                for x_t, var2 in zip(xs, var2s):
                    xn = pool.tile([128, 2, C], f32, tag="xn", name="xn")
                    layernorm_apply(xn, x_t, var2)
                    xns.append(xn)
                return {"pair": pair, "xs": xs, "xns": xns}

            def tp_qkv(st):
                """Transpose xn -> xnT and run QKV matmuls (PE-heavy)."""
                xns = st.pop("xns")
                xnT = pool.tile([128, 3, 2 * T], f32r, tag="xnT", name="xnT")
                transpose_pair(xnT, xns, "tp")

                qTs = [pool.tile([128, 3, T], f32r, tag="qT", name="qT")
                       for _ in range(2)]
                kTs = [pool.tile([128, 3, T], f32r, tag="kT", name="kT")
                       for _ in range(2)]
                for dsts, w, hb, hasb in ((qTs, wq, qb, has_qb),
                                          (kTs, wk, kb, has_kb)):
                    for m in range(3):
                        ps = ppool.tile([128, 2, T], f32, tag="ps", name="qk_ps")
                        flat = ps[:].rearrange("p a t -> p (a t)")
                        for c in range(3):
                            nc.tensor.matmul(flat, w[:, c, m * 128:(m + 1) * 128],
                                             xnT[:, c, :], start=(c == 0),
                                             stop=(c == 2 and not hasb))
                        if hasb:
                            nc.tensor.matmul(flat, hb[:, m * 128:(m + 1) * 128],
                                             ones[0:1, :], start=False, stop=True)
                        for bi in range(2):
                            nc.scalar.copy(dsts[bi][:, m, :], ps[:, bi])
                vs = [pool.tile([128, 2, H, D], f32r, tag="v", name="v")
                      for _ in range(2)]
                for bi in range(2):
                    for kt in range(2):
                        tk = 2 * bi + kt
                        ps = ppool.tile([128, C], f32, tag="ps", name="v_ps")
                        for c in range(3):
                            nc.tensor.matmul(
                                ps[:], xnT[:, c, tk * 128:(tk + 1) * 128],
                                wv[:, c, :], start=(c == 0),
                                stop=(c == 2 and not has_vb))
                        if has_vb:
                            nc.tensor.matmul(ps[:], ones[0:1, 0:128], vb,
                                             start=False, stop=True)
                        nc.vector.tensor_copy(
                            vs[bi][:, kt],
                            ps[:].rearrange("p (h d) -> p h d", d=D))
                st.update(qTs=qTs, kTs=kTs, vs=vs)
                return st

            def attention(st):
                """Causal attention per batch; attnT chunk ch holds heads
                (2ch, 2ch+1) on partitions (0..64, 64..128)."""
                qTs, kTs, vs = st["qTs"], st["kTs"], st["vs"]
                attnTs = [pool.tile([128, H, T], f32r, tag="attnT", name="attnT")
                          for _ in range(2)]
                weiTs = {}
                for bi in range(2):
                    for h in range(H):
                        po, ch = (h % 2) * 64, h // 2
                        sc = ppool.tile([128, 2, T], f32, tag="ps", name="sc")
                        for kt in range(2):
                            nc.tensor.matmul(sc[:, kt],
                                             kTs[bi][po:po + 64, ch,
                                                     kt * 128:(kt + 1) * 128],
                                             qTs[bi][po:po + 64, ch, :],
                                             start=True, stop=True)
                        weiT = pool.tile([128, 2, T], f32r, tag="weiT",
                                         name="weiT", bufs=9)
                        nc.scalar.activation(weiT[:], sc[:], AF.Exp, scale=ESC)
                        for kt in range(2):
                            nc.gpsimd.affine_select(
                                out=weiT[:, kt], in_=weiT[:, kt],
                                compare_op=ALU.is_ge, fill=0.0,
                                base=-(kt * 128), pattern=[[1, T]],
                                channel_multiplier=-1)
                        weiTs[(bi, h)] = weiT
                for bi in range(2):
                    for h in range(H):
                        weiT = weiTs[(bi, h)]
                        av = ppool.tile([64, T], f32, tag="ps", name="av")
                        sm = ppool.tile([64, T], f32, tag="ps", name="sm")
                        for kt in range(2):
                            nc.tensor.matmul(av[:], vs[bi][:, kt, h, :],
                                             weiT[:, kt], start=(kt == 0),
                                             stop=(kt == 1))
                        for kt in range(2):
                            nc.tensor.matmul(sm[:], ones[:, 0:64],
                                             weiT[:, kt], start=(kt == 0),
                                             stop=(kt == 1))
                        rcp = pool.tile([64, T], f32, tag="rcp", name="rcp",
                                        bufs=3)
                        nc.vector.reciprocal_approx_fast(rcp[:], sm[:])
                        nc.vector.tensor_tensor(attnTs[bi][0:D, h, :],
                                                av[:], rcp[:], ALU.mult)
                st["attnTs"] = attnTs

            def proj_stats(st):
                """proj + residual -> h, LN2 stats (PE: proj matmuls)."""
                attnTs, xs = st["attnTs"], st["xs"]
                hs, hvar2s = [], []
                for bi in range(2):
                    h_t = pool.tile([128, 2, C], f32, tag="h", name="h_t")
                    for kt in range(2):
                        ps = ppool.tile([128, C], f32, tag="ps", name="pr_ps")
                        for h in range(H):
                            nc.tensor.matmul(
                                ps[:], attnTs[bi][0:D, h, kt * 128:(kt + 1) * 128],
                                wp[0:64, h, :], start=(h == 0),
                                stop=(h == H - 1 and not has_bp))
                        if has_bp:
                            nc.tensor.matmul(ps[:], ones[0:1, 0:128], bpj,
                                             start=False, stop=True)
                        nc.vector.tensor_tensor(h_t[:, kt], ps[:], xs[bi][:, kt],
                                                ALU.add)
                    hs.append(h_t)
                    var2 = pool.tile([128, 2, 2], f32, tag="ln_hvar2",
                                     name="hvar2", bufs=4)
                    layernorm_stats(var2, h_t)
                    hvar2s.append(var2)
                st["hs"], st["hvar2s"] = hs, hvar2s

            def apply_hnT(st):
                """LN2 apply + hn transposes -> hnT."""
                hns = []
                for h_t, var2 in zip(st["hs"], st.pop("hvar2s")):
                    hn = pool.tile([128, 2, C], f32, tag="hn", name="hn")
                    layernorm_apply(hn, h_t, var2)
                    hns.append(hn)
                hnT = pool.tile([128, 3, 2 * T], f32r, tag="hnT", name="hnT")
                transpose_pair(hnT, hns, "tph")
                st["hnT"] = hnT

            def ffn(st):
                pair, hs, hnT = st["pair"], st["hs"], st["hnT"]
                h1T = pool.tile([128, 12, 2 * T], f32r, tag="h1T", name="h1T",
                                bufs=1)
                for mf in range(12):
                    ps = ppool.tile([128, 2, T], f32, tag="ps", name="f1_ps")
                    flat = ps[:].rearrange("p a t -> p (a t)")
                    for c in range(3):
                        nc.tensor.matmul(flat, w1[:, c, mf * 128:(mf + 1) * 128],
                                         hnT[:, c, :], start=(c == 0),
                                         stop=(c == 2))
                    nc.scalar.activation(h1T[:, mf, :], flat, AF.Relu,
                                         bias=b1[:, mf:mf + 1])
                for bi, b in enumerate(pair):
                    out_t = pool.tile([128, 2, C], f32, tag="out", name="out_t")
                    for kt in range(2):
                        tk = 2 * bi + kt
                        ps = ppool.tile([128, C], f32, tag="ps", name="f2_ps")
                        for mf in range(12):
                            nc.tensor.matmul(
                                ps[:], h1T[:, mf, tk * 128:(tk + 1) * 128],
                                w2[:, mf, :], start=(mf == 0),
                                stop=(mf == 11 and not has_b2))
                        if has_b2:
                            nc.tensor.matmul(ps[:], ones[0:1, 0:128], b2b,
                                             start=False, stop=True)
                        nc.vector.tensor_tensor(out_t[:, kt], ps[:],
                                                hs[bi][:, kt], ALU.add)
                        nc.sync.dma_start(out_d[b, kt * 128:(kt + 1) * 128, :],
                                          out_t[:, kt])

            # Software pipeline: the next pair's LN1 runs during this pair's
            # attention, and its transposes+QKV (PE work) fill the LN2 chain
            # latency, so the PE never idles long enough to re-throttle.
            NP = BPC // 2
            st = tp_qkv(ln1_load(0))
            for bp in range(NP):
                attention(st)
                nxt = ln1_load(bp + 1) if bp + 1 < NP else None
                proj_stats(st)
                if nxt is not None:
                    nxt = tp_qkv(nxt)
                apply_hnT(st)
                ffn(st)
                st = nxt

    nc.compile()
    return nc


def kernel(x, Wq, Wk, Wv, Wproj, bproj, W1, b1, W2, b2, ln1_g, ln1_b, ln2_g, ln2_b):
    x = np.asarray(x, dtype=np.float32)
    Wq = np.asarray(Wq, dtype=np.float32)
    Wk = np.asarray(Wk, dtype=np.float32)
    Wv = np.asarray(Wv, dtype=np.float32)
    Wproj = np.asarray(Wproj, dtype=np.float32)
    bproj = np.asarray(bproj, dtype=np.float32)
    W1 = np.asarray(W1, dtype=np.float32)
    b1 = np.asarray(b1, dtype=np.float32)
    W2 = np.asarray(W2, dtype=np.float32)
    b2 = np.asarray(b2, dtype=np.float32)
    ln1_g = np.asarray(ln1_g, dtype=np.float32)
    ln1_b = np.asarray(ln1_b, dtype=np.float32)
    ln2_g = np.asarray(ln2_g, dtype=np.float32)
    ln2_b = np.asarray(ln2_b, dtype=np.float32)

    # Fold LN gains into the consuming weights; LN biases become extra input-side
    # biases folded through the weights (zero for the standard init).
    wq_h = np.ascontiguousarray(Wq.transpose(1, 0, 2).reshape(C, C) * ln1_g[:, None])
    wk_h = np.ascontiguousarray(Wk.transpose(1, 0, 2).reshape(C, C) * ln1_g[:, None])
    wv_h = np.ascontiguousarray(Wv.transpose(1, 0, 2).reshape(C, C) * ln1_g[:, None])
    qb_h = ln1_b @ wq_h
    kb_h = ln1_b @ wk_h
    vb_h = ln1_b @ wv_h
    wp_h = np.ascontiguousarray(Wproj.reshape(H, D, C))
    w1_h = np.ascontiguousarray(W1 * ln2_g[:, None])
    b1_h = np.ascontiguousarray(b1 + ln2_b @ w1_h)
    w2_h = np.ascontiguousarray(W2)
    biases_h = np.ascontiguousarray(np.stack([qb_h, kb_h, vb_h, bproj, b2]))

    flags = tuple(bool(np.any(v)) for v in (qb_h, kb_h, vb_h, bproj, b2))
    if flags not in _CACHE:
        _CACHE[flags] = _build(flags)
    nc = _CACHE[flags]

    shared = {"wq": wq_h, "wk": wk_h, "wv": wv_h, "wp": wp_h,
              "w1": w1_h, "w2": w2_h, "b1": b1_h, "biases": biases_h}
    in_maps = [{"x": np.ascontiguousarray(x[c * BPC:(c + 1) * BPC]), **shared}
               for c in range(N_CORES)]

    res = run_bass_kernel_spmd(nc, in_maps, list(range(N_CORES)), trace=TRACE)
    if TRACE:
        kernel.last_results = res
    return np.concatenate([res.results[c]["out"] for c in range(N_CORES)], axis=0)
